# revision 7
# baseline (speedup 1.0000x reference)
"""Pairwise L2 distance kernel: x [4096,768], anchors [100,64,768] -> [4096,100,64].

Distributed over 8 TRN2 NeuronCores as a 2x4 grid: batch (4096) split in 2,
anchor index (6400) split in 4.  Each core computes a [2048,1600] output block
as sqrt(x2[b] + a2[j] - 2*x@A^T).

The x@A^T matmul runs in fp8e4m3 with DoubleRow (K=256 per pass, fp32 PSUM
accumulate).  Norms are computed on device: anchor squares via SWDGE
multiply-accumulate DMA (SBUF->SBUF copy then *=, zero vector-engine cost),
reduced+broadcast by an fp8-DoubleRow ones-matmul; x2 split between DVE
(scalar_tensor_tensor accum, bf16 2x) and ACT (Square accum).  Inputs are
chunked and interleaved so the PE main loop starts as soon as the first
k-pair of anchors and first m-chunk of x land.  Host does layout transforms
only (transpose, dtype cast, partition packing).
"""

import sys

import numpy as np

for _p in ("/opt/trn_rl_repo", "/root/.axon_site/_ro/trn_rl_repo"):
    if _p not in sys.path:
        sys.path.append(_p)

import ml_dtypes

import concourse.bass as bass
import concourse.tile as tile
from concourse import bacc, mybir
from concourse.bass import ts
from concourse.bass_utils import run_bass_kernel_spmd

B, C, A, E = 4096, 100, 64, 768
J = C * A                 # 6400 flattened anchors
RB, RJ = 2, 4             # batch groups x anchor groups = 8 cores
MB = B // RB              # 2048 batch rows per core
NJ = J // RJ              # 1600 anchor cols per core
KT = E // 128             # 6 contraction tiles of 128
K2 = KT // 2              # 3 DoubleRow k-pair passes
MT = MB // 128            # 16 m-tiles per core
XC = 4                    # xt / xo arrive in 4 chunks of 4 m-tiles
HW = NJ // 2              # 800: half-width epilogue/psum unit
N_CH = [(0, 512), (512, 288)]   # n-chunks within one 800 half

SQ2_DMA = False           # SWDGE mult-accum unsupported (walrus NCC_IBIR077)
X2_DVE = set(range(0, MT, 2))   # x2 alternates DVE / ACT per m-tile

FP8 = mybir.dt.float8e4
BF16 = mybir.dt.bfloat16
F32 = mybir.dt.float32
NP_FP8 = ml_dtypes.float8_e4m3
NP_BF16 = ml_dtypes.bfloat16
Alu = mybir.AluOpType
Act = mybir.ActivationFunctionType
DR = mybir.MatmulPerfMode.DoubleRow


def pack_rows(a2d: np.ndarray) -> np.ndarray:
    """[n*128, F] -> [128, n*F]: row r=k*128+p lands at partition p, block k.
    Makes each SBUF partition's data contiguous in DRAM."""
    n = a2d.shape[0] // 128
    return np.ascontiguousarray(
        a2d.reshape(n, 128, a2d.shape[1]).transpose(1, 0, 2).reshape(128, -1)
    )


def build_graph() -> bass.Bass:
    nc = bacc.Bacc(None, target_bir_lowering=False, debug=False, num_devices=8)
    at_ext = nc.declare_dram_parameter("at", [128, KT * NJ], FP8, isOutput=False)
    xt_ext = nc.declare_dram_parameter("xt", [128, XC * KT * 512], FP8, isOutput=False)
    xo_ext = nc.declare_dram_parameter("xo", [128, MT * E], BF16, isOutput=False)
    out_ext = nc.declare_dram_parameter("out", [MB, NJ], BF16, isOutput=True)

    at_r = at_ext[:].rearrange("p (k n) -> p k n", k=KT)
    xt_r = xt_ext[:].rearrange("p (c k b) -> p c k b", c=XC, k=KT)
    xo_r = xo_ext[:].rearrange("p (c m e) -> p c m e", c=XC, m=MT // XC)

    with tile.TileContext(nc) as tc:
        with (
            tc.tile_pool(name="big", bufs=1) as big,
            tc.tile_pool(name="atq", bufs=K2) as atq,
            tc.tile_pool(name="xtc", bufs=XC) as xtc,
            tc.tile_pool(name="xoc", bufs=XC) as xoc,
            tc.tile_pool(name="sqp", bufs=K2) as sqp,
            tc.tile_pool(name="x2p", bufs=MT) as x2p,
            tc.tile_pool(name="wk", bufs=6) as wk,
            tc.tile_pool(name="outs", bufs=4) as outs,
            tc.tile_pool(name="ring", bufs=3, space=bass.MemorySpace.PSUM) as ring,
            tc.tile_pool(name="aux", bufs=1, space=bass.MemorySpace.PSUM) as aux,
        ):
            # ACT table preload: sqrt_and_others holds both Sqrt and Square;
            # a first dummy Sqrt pulls the set in during the DMA head.
            dummy = big.tile([128, 1], F32)
            nc.vector.memset(dummy, 0.0)
            nc.scalar.activation(dummy, dummy, Act.Sqrt)

            # Input DMAs, availability-ordered: the first k-pair of anchors
            # plus the first m-chunk of x gate the first matmuls; the rest
            # stream behind them.
            at_q = [atq.tile([128, 2, NJ], FP8, tag="at", name=f"at{q}") for q in range(K2)]
            xt_c = [xtc.tile([128, KT, 512], FP8, tag="xt", name=f"xt{c}") for c in range(XC)]
            xo_c = [xoc.tile([128, MT // XC, E], BF16, tag="xo", name=f"xo{c}") for c in range(XC)]
            nc.sync.dma_start(out=at_q[0], in_=at_r[:, 0:2, :])
            nc.sync.dma_start(out=xt_c[0], in_=xt_r[:, 0])
            nc.sync.dma_start(out=at_q[1], in_=at_r[:, 2:4, :])
            nc.sync.dma_start(out=at_q[2], in_=at_r[:, 4:6, :])
            nc.sync.dma_start(out=xo_c[0], in_=xo_r[:, 0])
            nc.sync.dma_start(out=xt_c[1], in_=xt_r[:, 1])
            nc.sync.dma_start(out=xo_c[1], in_=xo_r[:, 1])
            nc.sync.dma_start(out=xt_c[2], in_=xt_r[:, 2])
            nc.sync.dma_start(out=xo_c[2], in_=xo_r[:, 2])
            nc.sync.dma_start(out=xt_c[3], in_=xt_r[:, 3])
            nc.sync.dma_start(out=xo_c[3], in_=xo_r[:, 3])

            ones_dr = big.tile([128, 2, 128], FP8)
            nc.vector.memset(ones_dr, -0.5)
            warm_src = big.tile([128, 512], BF16)
            nc.vector.memset(warm_src, 0.125)
            warm_w = big.tile([128, 64], BF16)
            nc.vector.memset(warm_w, 0.125)

            # PE warm-up in the aux psum while the first inputs land (HAM
            # un-throttle needs ~3.4us of sustained PE activity).
            warm_ps = aux.tile([128, HW], F32, tag="aux", name="warm_ps")
            for wi in range(10):
                nc.tensor.matmul(
                    warm_ps[:64, :512], warm_w, warm_src,
                    start=(wi == 0), stop=(wi == 9),
                )

            # sq2[q] = at[q]^2 in fp8, via SWDGE: plain SBUF->SBUF copy then
            # multiply-accumulate of the same source — zero DVE/ACT cost.
            sq2 = []
            for q in range(K2):
                s = sqp.tile([128, 2, NJ], FP8, tag="sq", name=f"sq{q}")
                if SQ2_DMA:
                    nc.gpsimd.dma_start(out=s, in_=at_q[q])
                    nc.gpsimd.dma_start(out=s, in_=at_q[q], accum_op=Alu.mult)
                elif q == 0:
                    # DVE is idle until the first xo chunk lands; q1/q2 go to
                    # ACT so DVE's steady-state add cadence stays under PE.
                    nc.vector.tensor_mul(s, at_q[q], at_q[q])
                else:
                    nc.scalar.activation(s, at_q[q], Act.Square)
                sq2.append(s)

            a2b = [None, None]  # -0.5*a2[j] per half, broadcast on partitions

            def emit_a2_half(h):
                ps = aux.tile([128, HW], F32, tag="aux", name=f"psa2_{h}")
                for c0, w in N_CH:
                    for q in range(K2):
                        nc.tensor.matmul(
                            ps[:, c0 : c0 + w],
                            ones_dr,
                            sq2[q][:, :, h * HW + c0 : h * HW + c0 + w],
                            start=(q == 0), stop=(q == K2 - 1),
                            perf_mode=DR,
                        )
                a2b[h] = wk.tile([128, HW], F32, tag="a2b", name=f"a2b{h}", bufs=2)
                nc.scalar.copy(a2b[h], ps)

            # x2[m] = sum(x^2) per batch row: DVE scalar_tensor_tensor with
            # accumulator (bf16 2x) for early m-tiles, ACT Square-accum for
            # the rest — keeps either engine under the PE cadence.
            xsq_d = wk.tile([128, E], BF16, tag="xsqd", name="xsqd", bufs=2)
            xsq_a = wk.tile([128, E], BF16, tag="xsqa", name="xsqa", bufs=2)
            x2s = []

            def emit_x2(m):
                x2 = x2p.tile([128, 1], F32, tag="x2", name=f"x2_{m}")
                xo_sl = xo_c[m // XC][:, m % XC, :]
                if m in X2_DVE:
                    nc.vector.scalar_tensor_tensor(
                        xsq_d, xo_sl, 0.0, xo_sl,
                        Alu.bypass, Alu.mult, accum_out=x2,
                    )
                else:
                    nc.scalar.activation(
                        xsq_a, xo_sl, Act.Square, accum_out=x2
                    )
                x2s.append(x2)

            # Main loop.  Ring psum of 3 [128,800] tiles; q-outer keeps one
            # LDWEIGHTS per (m,q).  a2 setup is traced between m1 and m2 so
            # the PE reaches it as soon as sq2 is ready.  m0/m1 free their
            # psum via an ACT copy and get a2b added later on DVE.
            EARLY = {0, 1}
            deferred = []
            for m in range(MT):
                emit_x2(m)
                pts = [
                    ring.tile([128, HW], F32, tag="ps", name=f"ps{m}_{h}")
                    for h in range(2)
                ]
                for q in range(K2):
                    lhsT = xt_c[m // XC][:, 2 * q : 2 * q + 2, ts(m % XC, 128)]
                    for h in range(2):
                        for c0, w in N_CH:
                            nc.tensor.matmul(
                                pts[h][:, c0 : c0 + w],
                                lhsT,
                                at_q[q][:, :, h * HW + c0 : h * HW + c0 + w],
                                start=(q == 0), stop=(q == K2 - 1),
                                perf_mode=DR,
                            )

                t = wk.tile([128, NJ], F32, tag="t", name=f"t{m}", bufs=4)
                out_t = outs.tile([128, NJ], BF16, tag="out", name=f"out{m}")
                if m in EARLY:
                    for h in range(2):
                        nc.scalar.copy(t[:, h * HW : (h + 1) * HW], pts[h])
                    deferred.append((m, t, out_t))
                else:
                    for h in range(2):
                        nc.vector.tensor_add(
                            t[:, h * HW : (h + 1) * HW], pts[h], a2b[h]
                        )
                    nc.scalar.activation(
                        out_t, t, Act.Sqrt, bias=x2s[m], scale=-2.0
                    )
                    if m == MT - 1:
                        for h in range(2):
                            nc.sync.dma_start(
                                out=out_ext[ts(m, 128), h * HW : (h + 1) * HW],
                                in_=out_t[:, h * HW : (h + 1) * HW],
                            )
                    else:
                        nc.sync.dma_start(out=out_ext[ts(m, 128), :], in_=out_t)

                if m == 1:
                    emit_a2_half(0)
                    emit_a2_half(1)
                    # a2b now exists: finish the early tiles (DVE in-place
                    # add, then the normal sqrt epilogue).
                    for em, et, eout in deferred:
                        for h in range(2):
                            nc.vector.tensor_add(
                                et[:, h * HW : (h + 1) * HW],
                                et[:, h * HW : (h + 1) * HW],
                                a2b[h],
                            )
                        nc.scalar.activation(
                            eout, et, Act.Sqrt, bias=x2s[em], scale=-2.0
                        )
                        nc.sync.dma_start(out=out_ext[ts(em, 128), :], in_=eout)

    nc.compile()
    return nc


def make_in_maps(x32: np.ndarray, a32: np.ndarray) -> list[dict[str, np.ndarray]]:
    xt_f8 = x32.T.astype(NP_FP8)           # [E, B]
    xo_bf = x32.astype(NP_BF16)            # [B, E]
    at_f8 = a32.T.astype(NP_FP8)           # [E, J]
    in_maps = []
    for c in range(8):
        g, h = c // RJ, c % RJ
        xt_p = pack_rows(xt_f8[:, g * MB : (g + 1) * MB])      # [128, 6*2048]
        xt_p = np.ascontiguousarray(
            xt_p.reshape(128, KT, XC, 512).transpose(0, 2, 1, 3)
        ).reshape(128, -1)                                      # chunk-major
        in_maps.append({
            "at": pack_rows(at_f8[:, h * NJ : (h + 1) * NJ]),
            "xt": xt_p,
            "xo": pack_rows(xo_bf[g * MB : (g + 1) * MB, :]),
        })
    return in_maps


def kernel(x: np.ndarray, anchors: np.ndarray) -> np.ndarray:
    x32 = np.asarray(x, dtype=np.float32)
    a32 = np.asarray(anchors, dtype=np.float32).reshape(J, E)

    nc = build_graph()
    in_maps = make_in_maps(x32, a32)
    results = run_bass_kernel_spmd(nc, in_maps, core_ids=list(range(8))).results

    out = np.empty((B, J), dtype=np.float32)
    for c in range(8):
        g, h = c // RJ, c % RJ
        out[g * MB : (g + 1) * MB, h * NJ : (h + 1) * NJ] = results[c][
            "out"
        ].astype(np.float32)
    return out.reshape(B, C, A)


# revision 16
# speedup vs baseline: 1.0318x; 1.0318x over previous
"""Pairwise L2 distance kernel: x [4096,768], anchors [100,64,768] -> [4096,100,64].

Distributed over 8 TRN2 NeuronCores as a 2x4 grid: batch (4096) split in 2,
anchor index (6400) split in 4.  Each core computes a [2048,1600] output block
as sqrt(x2[b] + a2[j] - 2*x@A^T).

The x@A^T matmul runs in fp8e4m3 with DoubleRow (K=256 per pass, fp32 PSUM
accumulate).  Norms are computed on device: anchor squares via SWDGE
multiply-accumulate DMA (SBUF->SBUF copy then *=, zero vector-engine cost),
reduced+broadcast by an fp8-DoubleRow ones-matmul; x2 split between DVE
(scalar_tensor_tensor accum, bf16 2x) and ACT (Square accum).  Inputs are
chunked and interleaved so the PE main loop starts as soon as the first
k-pair of anchors and first m-chunk of x land.  Host does layout transforms
only (transpose, dtype cast, partition packing).
"""

import sys

import numpy as np

for _p in ("/opt/trn_rl_repo", "/root/.axon_site/_ro/trn_rl_repo"):
    if _p not in sys.path:
        sys.path.append(_p)

import ml_dtypes

import concourse.bass as bass
import concourse.tile as tile
from concourse import bacc, mybir
from concourse.bass import ts
from concourse.bass_utils import run_bass_kernel_spmd

B, C, A, E = 4096, 100, 64, 768
J = C * A                 # 6400 flattened anchors
RB, RJ = 2, 4             # batch groups x anchor groups = 8 cores
MB = B // RB              # 2048 batch rows per core
NJ = J // RJ              # 1600 anchor cols per core
KT = E // 128             # 6 contraction tiles of 128
K2 = KT // 2              # 3 DoubleRow k-pair passes
MT = MB // 128            # 16 m-tiles per core
XC = 4                    # xt / xo arrive in 4 chunks of 4 m-tiles
HW = NJ // 2              # 800: half-width epilogue/psum unit
N_CH = [(0, 512), (512, 288)]   # n-chunks within one 800 half

SQ2_DMA = False           # SWDGE mult-accum unsupported (walrus NCC_IBIR077)
X2_DVE = set(range(0, MT, 2))   # x2 alternates DVE / ACT per m-tile

FP8 = mybir.dt.float8e4
BF16 = mybir.dt.bfloat16
F32 = mybir.dt.float32
NP_FP8 = ml_dtypes.float8_e4m3
NP_BF16 = ml_dtypes.bfloat16
Alu = mybir.AluOpType
Act = mybir.ActivationFunctionType
DR = mybir.MatmulPerfMode.DoubleRow


def pack_rows(a2d: np.ndarray) -> np.ndarray:
    """[n*128, F] -> [128, n*F]: row r=k*128+p lands at partition p, block k.
    Makes each SBUF partition's data contiguous in DRAM."""
    n = a2d.shape[0] // 128
    return np.ascontiguousarray(
        a2d.reshape(n, 128, a2d.shape[1]).transpose(1, 0, 2).reshape(128, -1)
    )


def build_graph() -> bass.Bass:
    nc = bacc.Bacc(None, target_bir_lowering=False, debug=False, num_devices=8)
    at_ext = nc.declare_dram_parameter("at", [128, KT * NJ], FP8, isOutput=False)
    xt_ext = nc.declare_dram_parameter("xt", [128, XC * KT * 512], FP8, isOutput=False)
    xo_ext = nc.declare_dram_parameter("xo", [128, MT * E], FP8, isOutput=False)
    out_ext = nc.declare_dram_parameter("out", [MB, NJ], BF16, isOutput=True)

    at_r = at_ext[:].rearrange("p (k n) -> p k n", k=KT)
    xt_r = xt_ext[:].rearrange("p (c k b) -> p c k b", c=XC, k=KT)
    xo_r = xo_ext[:].rearrange("p (c m e) -> p c m e", c=XC, m=MT // XC)

    with tile.TileContext(nc) as tc:
        with (
            tc.tile_pool(name="big", bufs=1) as big,
            tc.tile_pool(name="atq", bufs=K2) as atq,
            tc.tile_pool(name="xtc", bufs=XC) as xtc,
            tc.tile_pool(name="xoc", bufs=XC) as xoc,
            tc.tile_pool(name="sqp", bufs=K2) as sqp,
            tc.tile_pool(name="x2p", bufs=MT) as x2p,
            tc.tile_pool(name="wk", bufs=6) as wk,
            tc.tile_pool(name="outs", bufs=4) as outs,
            tc.tile_pool(name="ring", bufs=3, space=bass.MemorySpace.PSUM) as ring,
            tc.tile_pool(name="aux", bufs=1, space=bass.MemorySpace.PSUM) as aux,
        ):
            # ACT table preload: sqrt_and_others holds both Sqrt and Square;
            # a first dummy Sqrt pulls the set in during the DMA head.
            dummy = big.tile([128, 1], F32)
            nc.vector.memset(dummy, 0.0)
            nc.scalar.activation(dummy, dummy, Act.Sqrt)

            # Input DMAs, availability-ordered: the first k-pair of anchors
            # plus the first m-chunk of x gate the first matmuls; the rest
            # stream behind them.
            at_q = [atq.tile([128, 2, NJ], FP8, tag="at", name=f"at{q}") for q in range(K2)]
            xt_c = [xtc.tile([128, KT, 512], FP8, tag="xt", name=f"xt{c}") for c in range(XC)]
            xo_c = [xoc.tile([128, MT // XC, E], FP8, tag="xo", name=f"xo{c}") for c in range(XC)]
            nc.sync.dma_start(out=at_q[0], in_=at_r[:, 0:2, :])
            nc.sync.dma_start(out=xt_c[0], in_=xt_r[:, 0])
            nc.sync.dma_start(out=at_q[1], in_=at_r[:, 2:4, :])
            nc.sync.dma_start(out=at_q[2], in_=at_r[:, 4:6, :])
            nc.sync.dma_start(out=xo_c[0], in_=xo_r[:, 0])
            nc.sync.dma_start(out=xt_c[1], in_=xt_r[:, 1])
            nc.sync.dma_start(out=xo_c[1], in_=xo_r[:, 1])
            nc.sync.dma_start(out=xt_c[2], in_=xt_r[:, 2])
            nc.sync.dma_start(out=xo_c[2], in_=xo_r[:, 2])
            nc.sync.dma_start(out=xt_c[3], in_=xt_r[:, 3])
            nc.sync.dma_start(out=xo_c[3], in_=xo_r[:, 3])

            ones_dr = big.tile([128, 2, 128], FP8)
            nc.vector.memset(ones_dr, -0.5)
            warm_src = big.tile([128, 512], BF16)
            nc.vector.memset(warm_src, 0.125)
            warm_w = big.tile([128, 64], BF16)
            nc.vector.memset(warm_w, 0.125)

            # PE warm-up in the aux psum while the first inputs land (HAM
            # un-throttle needs ~3.4us of sustained PE activity).
            warm_ps = aux.tile([128, HW], F32, tag="aux", name="warm_ps")
            for wi in range(10):
                nc.tensor.matmul(
                    warm_ps[:64, :512], warm_w, warm_src,
                    start=(wi == 0), stop=(wi == 9),
                )

            # sq2[q] = at[q]^2 in fp8, via SWDGE: plain SBUF->SBUF copy then
            # multiply-accumulate of the same source — zero DVE/ACT cost.
            sq2 = []
            for q in range(K2):
                s = sqp.tile([128, 2, NJ], FP8, tag="sq", name=f"sq{q}")
                if SQ2_DMA:
                    nc.gpsimd.dma_start(out=s, in_=at_q[q])
                    nc.gpsimd.dma_start(out=s, in_=at_q[q], accum_op=Alu.mult)
                elif q == 1:
                    # DVE is idle during the input head, so it takes q0/q2;
                    # ACT (busy later with sqrts) only takes q1.
                    nc.scalar.activation(s, at_q[q], Act.Square)
                else:
                    nc.vector.tensor_mul(s, at_q[q], at_q[q])
                sq2.append(s)

            a2b = [None, None]  # -0.5*a2[j] per half, broadcast on partitions

            def emit_a2_half(h):
                ps = aux.tile([128, HW], F32, tag="aux", name=f"psa2_{h}")
                for c0, w in N_CH:
                    for q in range(K2):
                        nc.tensor.matmul(
                            ps[:, c0 : c0 + w],
                            ones_dr,
                            sq2[q][:, :, h * HW + c0 : h * HW + c0 + w],
                            start=(q == 0), stop=(q == K2 - 1),
                            perf_mode=DR,
                        )
                a2b[h] = wk.tile([128, HW], F32, tag="a2b", name=f"a2b{h}", bufs=2)
                nc.scalar.copy(a2b[h], ps)

            # x2[m] = sum(x^2) per batch row: DVE scalar_tensor_tensor with
            # accumulator (bf16 2x) for early m-tiles, ACT Square-accum for
            # the rest — keeps either engine under the PE cadence.
            xsq_d = wk.tile([128, E], FP8, tag="xsqd", name="xsqd", bufs=2)
            xsq_a = wk.tile([128, E], FP8, tag="xsqa", name="xsqa", bufs=2)
            x2s = {}

            def emit_x2(m):
                x2 = x2p.tile([128, 1], F32, tag="x2", name=f"x2_{m}")
                xo_sl = xo_c[m // XC][:, m % XC, :]
                if m in X2_DVE:
                    nc.vector.scalar_tensor_tensor(
                        xsq_d, xo_sl, 0.0, xo_sl,
                        Alu.bypass, Alu.mult, accum_out=x2,
                    )
                else:
                    nc.scalar.activation(
                        xsq_a, xo_sl, Act.Square, accum_out=x2
                    )
                x2s[m] = x2

            # Main loop.  Ring psum of 3 [128,800] tiles; q-outer keeps one
            # LDWEIGHTS per (m,q).  a2 setup is traced between m1 and m2 so
            # the PE reaches it as soon as sq2 is ready.  m0/m1 free their
            # psum via an ACT copy and get a2b added later on DVE.
            EARLY = {0, 1}
            deferred = []
            for m in range(MT):
                pts = [
                    ring.tile([128, HW], F32, tag="ps", name=f"ps{m}_{h}")
                    for h in range(2)
                ]
                for q in range(K2):
                    lhsT = xt_c[m // XC][:, 2 * q : 2 * q + 2, ts(m % XC, 128)]
                    for h in range(2):
                        for c0, w in N_CH:
                            nc.tensor.matmul(
                                pts[h][:, c0 : c0 + w],
                                lhsT,
                                at_q[q][:, :, h * HW + c0 : h * HW + c0 + w],
                                start=(q == 0), stop=(q == K2 - 1),
                                perf_mode=DR,
                            )

                t = wk.tile([128, NJ], F32, tag="t", name=f"t{m}", bufs=4)
                out_t = outs.tile([128, NJ], BF16, tag="out", name=f"out{m}")
                if m in EARLY:
                    for h in range(2):
                        nc.scalar.copy(t[:, h * HW : (h + 1) * HW], pts[h])
                    deferred.append((m, t, out_t))
                else:
                    # x2 emitted here (not at block top) so it sits between
                    # epilogue ops in its engine's FIFO instead of blocking
                    # earlier work behind its xo-chunk DMA dependency.
                    emit_x2(m)
                    for h in range(2):
                        nc.vector.tensor_add(
                            t[:, h * HW : (h + 1) * HW], pts[h], a2b[h]
                        )
                    nc.scalar.activation(
                        out_t, t, Act.Sqrt, bias=x2s[m], scale=-2.0
                    )
                    if m == MT - 1:
                        for h in range(2):
                            nc.sync.dma_start(
                                out=out_ext[ts(m, 128), h * HW : (h + 1) * HW],
                                in_=out_t[:, h * HW : (h + 1) * HW],
                            )
                    else:
                        nc.sync.dma_start(out=out_ext[ts(m, 128), :], in_=out_t)

                if m == 1:
                    emit_a2_half(0)
                    emit_a2_half(1)
                    # a2b now exists: finish the early tiles (DVE in-place
                    # add, then the normal sqrt epilogue).
                    for em, et, eout in deferred:
                        emit_x2(em)
                        for h in range(2):
                            nc.vector.tensor_add(
                                et[:, h * HW : (h + 1) * HW],
                                et[:, h * HW : (h + 1) * HW],
                                a2b[h],
                            )
                        nc.scalar.activation(
                            eout, et, Act.Sqrt, bias=x2s[em], scale=-2.0
                        )
                        nc.sync.dma_start(out=out_ext[ts(em, 128), :], in_=eout)

    nc.compile()
    return nc


def make_in_maps(x32: np.ndarray, a32: np.ndarray) -> list[dict[str, np.ndarray]]:
    xt_f8 = x32.T.astype(NP_FP8)           # [E, B]
    xo_bf = x32.astype(NP_FP8)             # [B, E]
    at_f8 = a32.T.astype(NP_FP8)           # [E, J]
    in_maps = []
    for c in range(8):
        g, h = c // RJ, c % RJ
        xt_p = pack_rows(xt_f8[:, g * MB : (g + 1) * MB])      # [128, 6*2048]
        xt_p = np.ascontiguousarray(
            xt_p.reshape(128, KT, XC, 512).transpose(0, 2, 1, 3)
        ).reshape(128, -1)                                      # chunk-major
        in_maps.append({
            "at": pack_rows(at_f8[:, h * NJ : (h + 1) * NJ]),
            "xt": xt_p,
            "xo": pack_rows(xo_bf[g * MB : (g + 1) * MB, :]),
        })
    return in_maps


def kernel(x: np.ndarray, anchors: np.ndarray) -> np.ndarray:
    x32 = np.asarray(x, dtype=np.float32)
    a32 = np.asarray(anchors, dtype=np.float32).reshape(J, E)

    nc = build_graph()
    in_maps = make_in_maps(x32, a32)
    results = run_bass_kernel_spmd(nc, in_maps, core_ids=list(range(8))).results

    out = np.empty((B, J), dtype=np.float32)
    for c in range(8):
        g, h = c // RJ, c % RJ
        out[g * MB : (g + 1) * MB, h * NJ : (h + 1) * NJ] = results[c][
            "out"
        ].astype(np.float32)
    return out.reshape(B, C, A)
